# revision 14
# baseline (speedup 1.0000x reference)
"""DSS-network GNN kernel for trn2 (8 NeuronCores), v2.

Graph-parallel over cores (contiguous node ranges), bf16 data plane.
Per layer:
  pass1: load x (bf16, deferred BN-affine+relu applied on load), compute
         z2 = x @ Wn per 128-node chunk (PE, bf16), stage node-major,
         AllGather z2 piecewise (8 slab-sized pieces pipelined behind
         compute); accumulate subgraph-sum for x_sum; write affined x.
  h2:    x_sum AllGather (small), orig-graph conv computed SHARDED (each
         core only its own 256 columns, As kept resident in SBUF bf16),
         BN2 stats via tiny AllReduce.
  pass2: per chunk: ONE batched indirect gather (B*128 rows, OOB pads
         skipped), one-hot scatter matmuls + root term accumulate in
         PSUM, write h1 (pre-BN) bf16; BN1 stats -> tiny AllReduce.
Readout fused: affine+relu on load of final h1, mean pools + MLP.
"""
import numpy as np

from concourse import bass, bacc, mybir, tile
from concourse.masks import make_identity

f32 = mybir.dt.float32
bf16 = mybir.dt.bfloat16
i32 = mybir.dt.int32
P = 128
EPS = 1e-5


class Cfg:
    def __init__(self, NC, G, N=128, EMB=128, L=4, TASKS=10, B=9):
        assert EMB == 128 and N == 128
        self.NC, self.G, self.N, self.EMB, self.L, self.TASKS = NC, G, N, EMB, L, TASKS
        self.S = G * N                   # total subgraphs == orig nodes
        self.T = G * N * N               # total batched nodes
        self.TP = self.T // NC           # nodes per core
        self.G_loc = G // NC             # graphs per core
        self.CH = self.TP // P           # dst-chunks per core
        self.SLAB = 4096                 # nodes per pass-1 slab == AG piece
        self.NSLAB = self.TP // self.SLAB
        self.CPS = self.SLAB // P        # chunks per slab
        self.B = B                       # gather blocks per chunk (host-set)
        self.NB = self.CH * B            # gather block columns per core
        self.SGL = self.G_loc * N        # own orig-node slots


def rep3(ap2d, b, inner, bcast_inner):
    """[P, b]-slice -> 3D AP: bcast_inner: [P, b, inner] with inner step 0
    (each column value repeated `inner` times); else iota-style [P, b, inner]
    with b step 0 (the 2d free dim repeated b times; ap2d must be [P, inner])."""
    pp = ap2d.ap[0]
    if bcast_inner:
        return bass.AP(ap2d.tensor, ap2d.offset, [pp, ap2d.ap[1][:], [0, inner]])
    else:
        return bass.AP(ap2d.tensor, ap2d.offset, [pp, [0, b], ap2d.ap[1][:]])


def build(cfg: Cfg, taps=False):
    nc = bacc.Bacc("TRN2", target_bir_lowering=False, debug=False,
                   num_devices=cfg.NC)
    L, TP, CH, B, NB, S, SGL, G_loc = (cfg.L, cfg.TP, cfg.CH, cfg.B, cfg.NB,
                                       cfg.S, cfg.SGL, cfg.G_loc)
    T, NC, TASKS = cfg.T, cfg.NC, cfg.TASKS
    SLAB, NSLAB, CPS = cfg.SLAB, cfg.NSLAB, cfg.CPS
    NT = S // P
    rg = [list(range(NC))]

    def din(name, shape, dt=f32):
        return nc.dram_tensor(name, shape, dt, kind="ExternalInput").ap()

    xT0 = din("xT0", [P, TP], bf16)
    gidx = din("gidx", [P, NB], i32)
    gdst = din("gdst", [P, NB], bf16)
    Asn = din("Asn", [S, SGL], bf16)          # As[:, own orig-node cols]
    Wr = din("Wr", [L * P, P], bf16); Wn = din("Wn", [L * P, P], bf16)
    Wrs = din("Wrs", [L * P, P], bf16); Wns = din("Wns", [L * P, P], bf16)
    bia = din("bia", [L * P, 1]); gam = din("gam", [L * P, 1]); bet = din("bet", [L * P, 1])
    bias_ = din("bias_", [L * P, 1]); gams = din("gams", [L * P, 1]); bets = din("bets", [L * P, 1])
    Wf1 = din("Wf1", [P, 2 * P]); bf1c = din("bf1c", [2 * P, 1])
    Wf2 = din("Wf2", [2 * P, TASKS]); bf2c = din("bf2c", [TASKS, 1])
    out = nc.dram_tensor("out", [G_loc, TASKS], f32, kind="ExternalOutput").ap()
    if taps:
        dz2 = nc.dram_tensor("dz2", [TP, P], bf16, kind="ExternalOutput").ap()
        dxs = nc.dram_tensor("dxs", [P, SGL], bf16, kind="ExternalOutput").ap()
        dh2 = nc.dram_tensor("dh2", [P, SGL], f32, kind="ExternalOutput").ap()
        dh1 = nc.dram_tensor("dh1", [P, TP], bf16, kind="ExternalOutput").ap()
        dcg = nc.dram_tensor("dcg", [P, SGL], bf16, kind="ExternalOutput").ap()
        dxa = nc.dram_tensor("dxa", [P, TP], bf16, kind="ExternalOutput").ap()
        dst_ = nc.dram_tensor("dst_", [P, 4], f32, kind="ExternalOutput").ap()

    with tile.TileContext(nc) as tc:
        with (
            tc.tile_pool(name="const", bufs=1) as cst,
            tc.tile_pool(name="wts", bufs=1) as wts,
            tc.tile_pool(name="prm", bufs=1) as prm,
            tc.tile_pool(name="io", bufs=2) as io,
            tc.tile_pool(name="xg", bufs=12) as xgp,
            tc.tile_pool(name="oh", bufs=4) as ohp,
            tc.tile_pool(name="sm", bufs=1) as sm,
            tc.tile_pool(name="ps_t", bufs=2, space="PSUM") as ps_t,
            tc.tile_pool(name="ps_m", bufs=4, space="PSUM") as ps_m,
            tc.tile_pool(name="ps_h", bufs=1, space="PSUM") as ps_h,
            tc.tile_pool(name="dram", bufs=1, space="DRAM") as dram,
        ):
            # ------- persistent DRAM -------
            z2_stage = dram.tile([TP, P], bf16)
            z2_fulls = [dram.tile([T, P], bf16, addr_space="Shared",
                                  name=f"z2_full_{i}") for i in range(L)]
            xsum_bounce = dram.tile([P, SGL], bf16)
            xsum_fulls = [dram.tile([NC * P, SGL], bf16, addr_space="Shared",
                                    name=f"xsum_full_{i}") for i in range(L)]
            h1_a = dram.tile([P, TP], bf16)
            h1_b = dram.tile([P, TP], bf16)
            x_aff = dram.tile([P, TP], bf16)
            st_bounce = dram.tile([P, 2], f32)
            st_reds = [dram.tile([P, 2], f32, addr_space="Shared",
                                 name=f"st_red_{i}") for i in range(2 * L)]

            # ------- static SBUF -------
            ident = cst.tile([P, P], f32)
            make_identity(nc, ident[:])
            iota_i = cst.tile([P, P], i32)
            nc.gpsimd.iota(iota_i[:], pattern=[[1, P]], base=0, channel_multiplier=0)
            iota_b = cst.tile([P, P], bf16)
            nc.vector.tensor_copy(out=iota_b[:], in_=iota_i[:])
            gidx_sb = cst.tile([P, NB], i32)
            nc.sync.dma_start(out=gidx_sb[:], in_=gidx[:, :])
            gdst_sb = cst.tile([P, NB], bf16)
            nc.sync.dma_start(out=gdst_sb[:], in_=gdst[:, :])
            As_sb = cst.tile([P, NT, SGL], bf16)
            nc.sync.dma_start(out=As_sb[:], in_=Asn[:, :].rearrange(
                "(t p) j -> p t j", p=P))
            eps_col = cst.tile([P, 1], f32)
            nc.vector.memset(eps_col[:], EPS)
            hsub = cst.tile([P, CH], f32)

            params = {}

            for li in range(L):
                z2_full, xsum_full = z2_fulls[li], xsum_fulls[li]
                wsl = slice(li * P, (li + 1) * P)
                Wn_t = wts.tile([P, P], bf16, tag="Wn_t")
                nc.sync.dma_start(out=Wn_t[:], in_=Wn[wsl, :])
                Wr_t = wts.tile([P, P], bf16, tag="Wr_t")
                nc.sync.dma_start(out=Wr_t[:], in_=Wr[wsl, :])
                Wns_t = wts.tile([P, P], bf16, tag="Wns_t")
                nc.sync.dma_start(out=Wns_t[:], in_=Wns[wsl, :])
                Wrs_t = wts.tile([P, P], bf16, tag="Wrs_t")
                nc.sync.dma_start(out=Wrs_t[:], in_=Wrs[wsl, :])
                vecs = {}
                for nm, src in (("b", bia), ("g", gam), ("be", bet),
                                ("bs", bias_), ("gs", gams), ("bes", bets)):
                    v = wts.tile([P, 1], f32, tag=f"v_{nm}")
                    nc.sync.dma_start(out=v[:], in_=src[wsl, :])
                    vecs[nm] = v

                h1_cur = (h1_a if li % 2 == 0 else h1_b)
                h1_prev = (h1_b if li % 2 == 0 else h1_a)

                # ---------- pass 1: z2 + x_sum + piecewise AllGather ----------
                xsum_acc = sm.tile([P, SGL], f32, tag="xsum_acc")
                nc.vector.memset(xsum_acc[:], 0.0)
                for sl in range(NSLAB):
                    gl = sl // (NSLAB // G_loc)
                    raw = io.tile([P, SLAB], bf16, tag="raw")
                    src = xT0 if li == 0 else h1_prev[:]
                    nc.sync.dma_start(out=raw[:],
                                      in_=src[:, sl * SLAB:(sl + 1) * SLAB])
                    if li > 0:
                        A1p, Cgp = params[li - 1]
                        xt = io.tile([P, SLAB], bf16, tag="xt")
                        nc.vector.tensor_scalar_mul(out=xt[:], in0=raw[:],
                                                    scalar1=A1p[:])
                        nc.vector.tensor_tensor(
                            out=xt[:].rearrange("p (a b) -> p a b", a=CPS),
                            in0=xt[:].rearrange("p (a b) -> p a b", a=CPS),
                            in1=rep3(Cgp[:, gl * P:(gl + 1) * P], CPS, P, False),
                            op=mybir.AluOpType.add)
                        nc.vector.tensor_scalar_max(out=xt[:], in0=xt[:],
                                                    scalar1=0.0)
                        nc.sync.dma_start(out=x_aff[:, sl * SLAB:(sl + 1) * SLAB],
                                          in_=xt[:])
                    else:
                        xt = raw
                    z2st = io.tile([P, CPS, P], bf16, tag="z2st")
                    for j in range(CPS):
                        pz = ps_t.tile([P, P], f32, tag="pz")
                        nc.tensor.matmul(out=pz[:], lhsT=xt[:, j * P:(j + 1) * P],
                                         rhs=Wn_t[:], start=True, stop=True)
                        nc.scalar.activation(out=z2st[:, j, :], in_=pz[:],
                                             func=mybir.ActivationFunctionType.Copy)
                    nc.sync.dma_start(
                        out=z2_stage[:].rearrange("(a j p) f -> p (a j) f",
                                                  p=P, j=CPS)[
                            :, sl * CPS:(sl + 1) * CPS, :],
                        in_=z2st[:])
                    red = sm.tile([P, P], f32, tag="red")
                    nc.vector.tensor_reduce(
                        out=red[:], in_=xt[:].rearrange("p (s n) -> p n s", s=CPS),
                        axis=mybir.AxisListType.X, op=mybir.AluOpType.add)
                    nc.vector.tensor_add(out=xsum_acc[:, gl * P:(gl + 1) * P],
                                         in0=xsum_acc[:, gl * P:(gl + 1) * P],
                                         in1=red[:])
                nc.gpsimd.collective_compute(
                    "AllGather", mybir.AluOpType.bypass, replica_groups=rg,
                    ins=[z2_stage[:]], outs=[z2_full[:]])
                if taps and li == 0:
                    nc.sync.dma_start(out=dz2[:, :], in_=z2_stage[:])

                # x_sum scaled (own cols, feature-major) + AllGather
                xsown = sm.tile([P, SGL], bf16, tag="xsown")
                nc.scalar.activation(out=xsown[:], in_=xsum_acc[:],
                                     func=mybir.ActivationFunctionType.Copy,
                                     scale=1.0 / cfg.N)
                nc.sync.dma_start(out=xsum_bounce[:, :], in_=xsown[:])
                nc.gpsimd.collective_compute(
                    "AllGather", mybir.AluOpType.bypass, replica_groups=rg,
                    ins=[xsum_bounce[:]], outs=[xsum_full[:]])

                # ---------- h2 path (own columns only) ----------
                xsT = sm.tile([P, NC, SGL], bf16, tag="xsT")
                nc.sync.dma_start(out=xsT[:], in_=xsum_full[:].rearrange(
                    "(k p) j -> p k j", p=P))
                zs = sm.tile([P, NT, P], bf16, tag="zs")
                for t in range(NT):
                    kt, ht = t // G_loc, t % G_loc
                    pz = ps_t.tile([P, P], f32, tag="pz")
                    nc.tensor.matmul(out=pz[:],
                                     lhsT=xsT[:, kt, ht * P:(ht + 1) * P],
                                     rhs=Wns_t[:], start=True, stop=True)
                    nc.scalar.activation(out=zs[:, t, :], in_=pz[:],
                                         func=mybir.ActivationFunctionType.Copy)
                ph = ps_h.tile([P, SGL], f32, tag="ph")
                nc.tensor.matmul(out=ph[:], lhsT=Wrs_t[:], rhs=xsown[:],
                                 start=True, stop=False)
                for t in range(NT):
                    nc.tensor.matmul(out=ph[:], lhsT=zs[:, t, :],
                                     rhs=As_sb[:, t, :],
                                     start=False, stop=(t == NT - 1))
                h2pre = sm.tile([P, SGL], f32, tag="h2pre")
                nc.scalar.activation(out=h2pre[:], in_=ph[:],
                                     func=mybir.ActivationFunctionType.Identity,
                                     bias=vecs["bs"][:])
                # BN2 stats (partial over own cols) -> AllReduce
                stv2 = sm.tile([P, 2], f32, tag="stv2")
                nc.vector.tensor_reduce(out=stv2[:, 0:1], in_=h2pre[:],
                                        axis=mybir.AxisListType.X,
                                        op=mybir.AluOpType.add)
                sq2 = sm.tile([P, SGL], f32, tag="sq2")
                nc.vector.tensor_tensor(out=sq2[:], in0=h2pre[:], in1=h2pre[:],
                                        op=mybir.AluOpType.mult)
                nc.vector.tensor_reduce(out=stv2[:, 1:2], in_=sq2[:],
                                        axis=mybir.AxisListType.X,
                                        op=mybir.AluOpType.add)
                nc.sync.dma_start(out=st_bounce[:], in_=stv2[:])
                nc.gpsimd.collective_compute(
                    "AllReduce", mybir.AluOpType.add, replica_groups=rg,
                    ins=[st_bounce[:]], outs=[st_reds[2 * li][:]])
                str2 = sm.tile([P, 2], f32, tag="str2")
                nc.sync.dma_start(out=str2[:], in_=st_reds[2 * li][:])

                def bn_coeffs(str_, n, gamma_v, beta_v, tag):
                    mu = sm.tile([P, 1], f32, tag=f"mu{tag}")
                    nc.vector.tensor_scalar_mul(out=mu[:], in0=str_[:, 0:1],
                                                scalar1=1.0 / n)
                    ex = sm.tile([P, 1], f32, tag=f"ex{tag}")
                    nc.vector.tensor_scalar_mul(out=ex[:], in0=str_[:, 1:2],
                                                scalar1=1.0 / n)
                    mus = sm.tile([P, 1], f32, tag=f"mus{tag}")
                    nc.vector.tensor_tensor(out=mus[:], in0=mu[:], in1=mu[:],
                                            op=mybir.AluOpType.mult)
                    var = sm.tile([P, 1], f32, tag=f"var{tag}")
                    nc.vector.tensor_tensor(out=var[:], in0=ex[:], in1=mus[:],
                                            op=mybir.AluOpType.subtract)
                    sd = sm.tile([P, 1], f32, tag=f"sd{tag}")
                    nc.scalar.activation(out=sd[:], in_=var[:],
                                         func=mybir.ActivationFunctionType.Sqrt,
                                         bias=eps_col[:])
                    rstd = sm.tile([P, 1], f32, tag=f"rstd{tag}")
                    nc.vector.reciprocal(out=rstd[:], in_=sd[:])
                    A = sm.tile([P, 1], f32, tag=f"A{tag}")
                    nc.vector.tensor_tensor(out=A[:], in0=gamma_v[:], in1=rstd[:],
                                            op=mybir.AluOpType.mult)
                    muA = sm.tile([P, 1], f32, tag=f"muA{tag}")
                    nc.vector.tensor_tensor(out=muA[:], in0=mu[:], in1=A[:],
                                            op=mybir.AluOpType.mult)
                    C = sm.tile([P, 1], f32, tag=f"C{tag}")
                    nc.vector.tensor_tensor(out=C[:], in0=beta_v[:], in1=muA[:],
                                            op=mybir.AluOpType.subtract)
                    return A, C

                A2, C2 = bn_coeffs(str2, S, vecs["gs"], vecs["bes"], "2")
                h2own = sm.tile([P, SGL], f32, tag="h2own")
                nc.vector.tensor_scalar(out=h2own[:], in0=h2pre[:],
                                        scalar1=A2[:], scalar2=C2[:],
                                        op0=mybir.AluOpType.mult,
                                        op1=mybir.AluOpType.add)
                if taps and li == 0:
                    nc.sync.dma_start(out=dxs[:, :], in_=xsum_bounce[:])
                    nc.sync.dma_start(out=dh2[:, :], in_=h2own[:])
                    nc.sync.dma_start(out=dst_[:, 0:2], in_=str2[:])

                # ---------- pass 2: batched gather + scatter ----------
                ssum = sm.tile([P, NSLAB], f32, tag="ssum")
                ssq = sm.tile([P, NSLAB], f32, tag="ssq")
                x2src = xT0 if li == 0 else x_aff[:]
                for sl in range(NSLAB):
                    xa = io.tile([P, SLAB], bf16, tag="xa")
                    nc.sync.dma_start(out=xa[:],
                                      in_=x2src[:, sl * SLAB:(sl + 1) * SLAB])
                    h1st = io.tile([P, CPS, P], bf16, tag="h1st")
                    for j in range(CPS):
                        c = sl * CPS + j
                        oh = ohp.tile([P, B, P], bf16, tag="oh")
                        nc.vector.tensor_tensor(
                            out=oh[:],
                            in0=rep3(gdst_sb[:, c * B:(c + 1) * B], B, P, True),
                            in1=rep3(iota_b[:], B, P, False),
                            op=mybir.AluOpType.is_equal)
                        pm = ps_m.tile([P, P], f32, tag="pm")
                        nc.tensor.matmul(out=pm[:], lhsT=Wr_t[:],
                                         rhs=xa[:, j * P:(j + 1) * P],
                                         start=True, stop=False)
                        for b in range(B):
                            xg = xgp.tile([P, P], bf16, tag="xg")
                            nc.gpsimd.indirect_dma_start(
                                out=xg[:], out_offset=None, in_=z2_full[:],
                                in_offset=bass.IndirectOffsetOnAxis(
                                    ap=gidx_sb[:, c * B + b:c * B + b + 1],
                                    axis=0))
                            nc.tensor.matmul(out=pm[:],
                                             lhsT=xg[:], rhs=oh[:, b, :],
                                             start=False, stop=(b == B - 1))
                        nc.scalar.activation(out=h1st[:, j, :], in_=pm[:],
                                             func=mybir.ActivationFunctionType.Identity,
                                             bias=vecs["b"][:])
                    nc.sync.dma_start(out=h1_cur[:, sl * SLAB:(sl + 1) * SLAB],
                                      in_=h1st[:].rearrange("p a b -> p (a b)"))
                    rs = sm.tile([P, 1], f32, tag="rs")
                    nc.vector.tensor_reduce(
                        out=rs[:], in_=h1st[:].rearrange("p a b -> p (a b)"),
                        axis=mybir.AxisListType.X, op=mybir.AluOpType.add)
                    nc.vector.tensor_copy(out=ssum[:, sl:sl + 1], in_=rs[:])
                    sqt = io.tile([P, SLAB], bf16, tag="sqt")
                    nc.vector.tensor_tensor(
                        out=sqt[:], in0=h1st[:].rearrange("p a b -> p (a b)"),
                        in1=h1st[:].rearrange("p a b -> p (a b)"),
                        op=mybir.AluOpType.mult)
                    rq = sm.tile([P, 1], f32, tag="rq")
                    nc.vector.tensor_reduce(out=rq[:], in_=sqt[:],
                                            axis=mybir.AxisListType.X,
                                            op=mybir.AluOpType.add)
                    nc.vector.tensor_copy(out=ssq[:, sl:sl + 1], in_=rq[:])

                # ---------- BN1 stats AllReduce + next-layer params ----------
                stv1 = sm.tile([P, 2], f32, tag="stv1")
                nc.vector.tensor_reduce(out=stv1[:, 0:1], in_=ssum[:],
                                        axis=mybir.AxisListType.X,
                                        op=mybir.AluOpType.add)
                nc.vector.tensor_reduce(out=stv1[:, 1:2], in_=ssq[:],
                                        axis=mybir.AxisListType.X,
                                        op=mybir.AluOpType.add)
                nc.sync.dma_start(out=st_bounce[:], in_=stv1[:])
                nc.gpsimd.collective_compute(
                    "AllReduce", mybir.AluOpType.add, replica_groups=rg,
                    ins=[st_bounce[:]], outs=[st_reds[2 * li + 1][:]])
                str1 = sm.tile([P, 2], f32, tag="str1")
                nc.sync.dma_start(out=str1[:], in_=st_reds[2 * li + 1][:])
                A1, sh1 = bn_coeffs(str1, T, vecs["g"], vecs["be"], "1")
                A1_sb = prm.tile([P, 1], f32, tag=f"A1_{li}")
                nc.vector.tensor_copy(out=A1_sb[:], in_=A1[:])
                Cg_sb = prm.tile([P, SGL], bf16, tag=f"Cg_{li}")
                nc.vector.tensor_scalar_add(out=Cg_sb[:], in0=h2own[:],
                                            scalar1=sh1[:])
                params[li] = (A1_sb, Cg_sb)
                if taps and li == 0:
                    nc.sync.dma_start(out=dh1[:, :], in_=h1_cur[:])
                    nc.sync.dma_start(out=dcg[:, :], in_=Cg_sb[:])
                    nc.sync.dma_start(out=dst_[:, 2:4], in_=str1[:])
                if taps and li == 1:
                    nc.sync.dma_start(out=dxa[:, :], in_=x_aff[:])

            # ---------- readout ----------
            A1f, Cgf = params[L - 1]
            h1_fin = (h1_a if (L - 1) % 2 == 0 else h1_b)
            for sl in range(NSLAB):
                gl = sl // (NSLAB // G_loc)
                raw = io.tile([P, SLAB], bf16, tag="raw")
                nc.sync.dma_start(out=raw[:],
                                  in_=h1_fin[:, sl * SLAB:(sl + 1) * SLAB])
                xt = io.tile([P, SLAB], bf16, tag="xt")
                nc.vector.tensor_scalar_mul(out=xt[:], in0=raw[:], scalar1=A1f[:])
                nc.vector.tensor_tensor(
                    out=xt[:].rearrange("p (a b) -> p a b", a=CPS),
                    in0=xt[:].rearrange("p (a b) -> p a b", a=CPS),
                    in1=rep3(Cgf[:, gl * P:(gl + 1) * P], CPS, P, False),
                    op=mybir.AluOpType.add)
                nc.vector.tensor_scalar_max(out=xt[:], in0=xt[:], scalar1=0.0)
                nc.vector.tensor_reduce(
                    out=hsub[:, sl * CPS:(sl + 1) * CPS],
                    in_=xt[:].rearrange("p (c n) -> p c n", c=CPS),
                    axis=mybir.AxisListType.X, op=mybir.AluOpType.add)
            hg = sm.tile([P, G_loc], f32, tag="hg")
            nc.vector.tensor_reduce(
                out=hg[:], in_=hsub[:].rearrange("p (g s) -> p g s", g=G_loc),
                axis=mybir.AxisListType.X, op=mybir.AluOpType.add)
            nc.vector.tensor_scalar_mul(out=hg[:], in0=hg[:],
                                        scalar1=1.0 / (cfg.N * cfg.N))
            Wf1_sb = sm.tile([P, 2 * P], f32, tag="Wf1_sb")
            nc.sync.dma_start(out=Wf1_sb[:], in_=Wf1[:, :])
            Wf2_sb = sm.tile([P, 2, TASKS], f32, tag="Wf2_sb")
            nc.sync.dma_start(out=Wf2_sb[:],
                              in_=Wf2[:, :].rearrange("(a p) t -> p a t", p=P))
            bf1_sb = sm.tile([P, 2], f32, tag="bf1_sb")
            nc.sync.dma_start(out=bf1_sb[:],
                              in_=bf1c[:, :].rearrange("(a p) o -> p (a o)", p=P))
            bf2_sb = sm.tile([TASKS, 1], f32, tag="bf2_sb")
            nc.sync.dma_start(out=bf2_sb[:], in_=bf2c[:, :])
            o1 = sm.tile([P, 2, G_loc], f32, tag="o1")
            for h in range(2):
                p1 = ps_t.tile([P, G_loc], f32, tag="pz")
                nc.tensor.matmul(out=p1[:], lhsT=Wf1_sb[:, h * P:(h + 1) * P],
                                 rhs=hg[:], start=True, stop=True)
                nc.scalar.activation(out=o1[:, h, :], in_=p1[:],
                                     func=mybir.ActivationFunctionType.Relu,
                                     bias=bf1_sb[:, h:h + 1])
            p2 = ps_t.tile([TASKS, G_loc], f32, tag="pz")
            for h in range(2):
                nc.tensor.matmul(out=p2[:], lhsT=Wf2_sb[:, h, :], rhs=o1[:, h, :],
                                 start=(h == 0), stop=(h == 1))
            oT = sm.tile([TASKS, G_loc], f32, tag="oT")
            nc.scalar.activation(out=oT[:], in_=p2[:],
                                 func=mybir.ActivationFunctionType.Identity,
                                 bias=bf2_sb[:])
            nc.sync.dma_start(out=out[:, :].rearrange("a b -> b a"), in_=oT[:])

    nc.compile()
    return nc


def host_prep(inputs, cfg: Cfg):
    """Full inputs dict -> in_maps list per core."""
    import ml_dtypes
    bf = ml_dtypes.bfloat16
    NC, G, N, L, S, T, TP = cfg.NC, cfg.G, cfg.N, cfg.L, cfg.S, cfg.T, cfg.TP
    x = np.asarray(inputs["x"], np.float32)
    ei = np.asarray(inputs["edge_index"])
    oei = np.asarray(inputs["original_edge_index"])
    batch = np.asarray(inputs["batch"])
    sni = np.asarray(inputs["subgraph_node_idx"])
    sb = np.asarray(inputs["subgraph_batch"])
    nnps = np.asarray(inputs["num_nodes_per_subgraph"])
    sib = np.asarray(inputs["subgraph_idx_batch"])

    # verify the structured DSS layout this kernel hardcodes
    assert np.array_equal(batch, np.repeat(np.arange(G), N * N))
    assert np.array_equal(sni, np.tile(np.arange(N), S))
    assert np.array_equal(sb, np.repeat(np.arange(S), N))
    assert np.all(nnps == N)
    assert np.array_equal(sib, np.repeat(np.arange(G), N))

    As = np.zeros((S, S), np.float32)
    np.add.at(As, (oei[0], oei[1]), 1.0)

    src, dst = ei[0].astype(np.int64), ei[1].astype(np.int64)
    core = dst // TP
    chunk_gl = dst // P
    cnt = np.bincount(chunk_gl, minlength=T // P)
    B = max(1, int(np.ceil(cnt.max() / P)))
    cfg.B = B
    cfg.NB = cfg.CH * B

    src_row = src        # z2_full row == global node id (rank-concat layout)

    def stack(w):
        return np.asarray(w, np.float32).reshape(L * 128, 128)

    def col(v):
        return np.asarray(v, np.float32).reshape(L * 128, 1)

    common = dict(
        Wr=stack(inputs["Wr"]).astype(bf), Wn=stack(inputs["Wn"]).astype(bf),
        Wrs=stack(inputs["Wr_s"]).astype(bf), Wns=stack(inputs["Wn_s"]).astype(bf),
        bia=col(inputs["b"]), gam=col(inputs["gamma"]), bet=col(inputs["beta"]),
        bias_=col(inputs["b_s"]), gams=col(inputs["gamma_s"]),
        bets=col(inputs["beta_s"]),
        Wf1=np.asarray(inputs["Wf1"], np.float32),
        bf1c=np.asarray(inputs["bf1"], np.float32).reshape(2 * 128, 1),
        Wf2=np.asarray(inputs["Wf2"], np.float32),
        bf2c=np.asarray(inputs["bf2"], np.float32).reshape(cfg.TASKS, 1),
    )

    in_maps = []
    for k in range(NC):
        sel = core == k
        s_k = src_row[sel]
        d_k = dst[sel] - k * TP
        order = np.argsort(d_k, kind="stable")
        s_k, d_k = s_k[order], d_k[order]
        ch = d_k // P
        cnt_k = np.bincount(ch, minlength=cfg.CH)
        starts = np.concatenate([[0], np.cumsum(cnt_k)])[:-1]
        pos_in_chunk = np.arange(len(d_k)) - starts[ch]
        slot = ch * (B * P) + pos_in_chunk
        gi = np.zeros(cfg.CH * B * P, np.int32)         # pad = row 0 (harmless)
        gd = np.full(cfg.CH * B * P, -1.0, np.float32)
        gi[slot] = s_k
        gd[slot] = (d_k % P).astype(np.float32)
        gi = gi.reshape(cfg.NB, P).T.copy()
        gd = gd.reshape(cfg.NB, P).T.copy().astype(bf)

        m = dict(common)
        m.update(
            xT0=np.ascontiguousarray(x[k * TP:(k + 1) * TP].T).astype(bf),
            gidx=gi, gdst=gd,
            Asn=np.ascontiguousarray(
                As[:, k * cfg.SGL:(k + 1) * cfg.SGL]).astype(bf),
        )
        in_maps.append(m)
    return in_maps


_CACHE = {}


def kernel(**inputs):
    """Full (unsharded) inputs -> full [G, TASKS] output, computed on 8
    trn2 NeuronCores via bass."""
    from concourse import bass_utils

    G = int(np.asarray(inputs["num_nodes_per_subgraph"]).shape[0])
    N = int(np.asarray(inputs["num_nodes_per_subgraph"])[0])
    TASKS = int(np.asarray(inputs["bf2"]).shape[0])
    L = int(np.asarray(inputs["Wr"]).shape[0])
    NC = 8

    cfg = Cfg(NC=NC, G=G, N=N, L=L, TASKS=TASKS)
    in_maps = host_prep(inputs, cfg)

    key = (NC, G, N, L, TASKS, cfg.B)
    if key not in _CACHE:
        _CACHE[key] = build(cfg)
    nc = _CACHE[key]

    res = bass_utils.run_bass_kernel_spmd(
        nc, in_maps, core_ids=list(range(NC)), trace=False)
    out = np.concatenate([res.results[k]["out"] for k in range(NC)], axis=0)
    return out.astype(np.float32)


# revision 21
# speedup vs baseline: 1.2358x; 1.2358x over previous
"""DSS-network GNN kernel for trn2 (8 NeuronCores), v2.

Graph-parallel over cores (contiguous node ranges), bf16 data plane.
Per layer:
  pass1: load x (bf16, deferred BN-affine+relu applied on load), compute
         z2 = x @ Wn per 128-node chunk (PE, bf16), stage node-major,
         AllGather z2 piecewise (8 slab-sized pieces pipelined behind
         compute); accumulate subgraph-sum for x_sum; write affined x.
  h2:    x_sum AllGather (small), orig-graph conv computed SHARDED (each
         core only its own 256 columns, As kept resident in SBUF bf16),
         BN2 stats via tiny AllReduce.
  pass2: per chunk: ONE batched indirect gather (B*128 rows, OOB pads
         skipped), one-hot scatter matmuls + root term accumulate in
         PSUM, write h1 (pre-BN) bf16; BN1 stats -> tiny AllReduce.
Readout fused: affine+relu on load of final h1, mean pools + MLP.
"""
import numpy as np

from concourse import bass, bacc, mybir, tile
from concourse.masks import make_identity

f32 = mybir.dt.float32
bf16 = mybir.dt.bfloat16
i32 = mybir.dt.int32
P = 128
EPS = 1e-5


class Cfg:
    def __init__(self, NC, G, N=128, EMB=128, L=4, TASKS=10, B=9):
        assert EMB == 128 and N == 128
        self.NC, self.G, self.N, self.EMB, self.L, self.TASKS = NC, G, N, EMB, L, TASKS
        self.S = G * N                   # total subgraphs == orig nodes
        self.T = G * N * N               # total batched nodes
        self.TP = self.T // NC           # nodes per core
        self.G_loc = G // NC             # graphs per core
        self.CH = self.TP // P           # dst-chunks per core
        self.SLAB = 4096                 # nodes per pass-1 slab == AG piece
        self.NSLAB = self.TP // self.SLAB
        self.CPS = self.SLAB // P        # chunks per slab
        self.B = B                       # gather blocks per chunk (host-set)
        self.NB = self.CH * B            # gather block columns per core
        self.SGL = self.G_loc * N        # own orig-node slots


def rep3(ap2d, b, inner, bcast_inner):
    """[P, b]-slice -> 3D AP: bcast_inner: [P, b, inner] with inner step 0
    (each column value repeated `inner` times); else iota-style [P, b, inner]
    with b step 0 (the 2d free dim repeated b times; ap2d must be [P, inner])."""
    pp = ap2d.ap[0]
    if bcast_inner:
        return bass.AP(ap2d.tensor, ap2d.offset, [pp, ap2d.ap[1][:], [0, inner]])
    else:
        return bass.AP(ap2d.tensor, ap2d.offset, [pp, [0, b], ap2d.ap[1][:]])


def build(cfg: Cfg, taps=False):
    nc = bacc.Bacc("TRN2", target_bir_lowering=False, debug=False,
                   num_devices=cfg.NC)
    L, TP, CH, B, NB, S, SGL, G_loc = (cfg.L, cfg.TP, cfg.CH, cfg.B, cfg.NB,
                                       cfg.S, cfg.SGL, cfg.G_loc)
    T, NC, TASKS = cfg.T, cfg.NC, cfg.TASKS
    SLAB, NSLAB, CPS = cfg.SLAB, cfg.NSLAB, cfg.CPS
    NT = S // P
    rg = [list(range(NC))]

    def din(name, shape, dt=f32):
        return nc.dram_tensor(name, shape, dt, kind="ExternalInput").ap()

    NHS = CH // 16                        # halfslab groups for dma_gather
    B_OV = B - 8                          # overflow blocks per chunk
    xT0 = din("xT0", [P, TP], bf16)
    g16 = din("g16", [32, NHS * 8 * P], mybir.dt.int16)
    govf = din("govf", [P, CH * B_OV], i32)
    gdst = din("gdst", [P, NB], bf16)
    Asn = din("Asn", [S, SGL], bf16)          # As[:, own orig-node cols]
    Wr = din("Wr", [L * P, P], bf16); Wn = din("Wn", [L * P, P], bf16)
    Wrs = din("Wrs", [L * P, P], bf16); Wns = din("Wns", [L * P, P], bf16)
    bia = din("bia", [L * P, 1]); gam = din("gam", [L * P, 1]); bet = din("bet", [L * P, 1])
    bias_ = din("bias_", [L * P, 1]); gams = din("gams", [L * P, 1]); bets = din("bets", [L * P, 1])
    Wf1 = din("Wf1", [P, 2 * P]); bf1c = din("bf1c", [2 * P, 1])
    Wf2 = din("Wf2", [2 * P, TASKS]); bf2c = din("bf2c", [TASKS, 1])
    out = nc.dram_tensor("out", [G_loc, TASKS], f32, kind="ExternalOutput").ap()
    if taps:
        dz2 = nc.dram_tensor("dz2", [TP, P], bf16, kind="ExternalOutput").ap()
        dxs = nc.dram_tensor("dxs", [P, SGL], bf16, kind="ExternalOutput").ap()
        dh2 = nc.dram_tensor("dh2", [P, SGL], f32, kind="ExternalOutput").ap()
        dh1 = nc.dram_tensor("dh1", [P, TP], bf16, kind="ExternalOutput").ap()
        dcg = nc.dram_tensor("dcg", [P, SGL], bf16, kind="ExternalOutput").ap()
        dxa = nc.dram_tensor("dxa", [P, TP], bf16, kind="ExternalOutput").ap()
        dst_ = nc.dram_tensor("dst_", [P, 4], f32, kind="ExternalOutput").ap()

    with tile.TileContext(nc) as tc:
        with (
            tc.tile_pool(name="const", bufs=1) as cst,
            tc.tile_pool(name="wts", bufs=1) as wts,
            tc.tile_pool(name="prm", bufs=1) as prm,
            tc.tile_pool(name="io", bufs=2) as io,
            tc.tile_pool(name="xg", bufs=6) as xgp,
            tc.tile_pool(name="xgs", bufs=1) as xsp,
            tc.tile_pool(name="oh", bufs=4) as ohp,
            tc.tile_pool(name="sm", bufs=1) as sm,
            tc.tile_pool(name="ps_t", bufs=2, space="PSUM") as ps_t,
            tc.tile_pool(name="ps_m", bufs=4, space="PSUM") as ps_m,
            tc.tile_pool(name="ps_h", bufs=1, space="PSUM") as ps_h,
            tc.tile_pool(name="dram", bufs=1, space="DRAM") as dram,
        ):
            # ------- persistent DRAM -------
            z2_stage = dram.tile([TP, P], bf16)
            z2_fulls = [dram.tile([T, P], bf16, addr_space="Shared",
                                  name=f"z2_full_{i}") for i in range(L)]
            xsum_bounce = dram.tile([P, SGL], bf16)
            xsum_fulls = [dram.tile([NC * P, SGL], bf16, addr_space="Shared",
                                    name=f"xsum_full_{i}") for i in range(L)]
            h1_a = dram.tile([P, TP], bf16)
            h1_b = dram.tile([P, TP], bf16)
            x_aff = dram.tile([P, TP], bf16)
            st_bounce = dram.tile([P, 2], f32)
            st_reds = [dram.tile([P, 2], f32, addr_space="Shared",
                                 name=f"st_red_{i}") for i in range(2 * L)]

            # ------- static SBUF -------
            ident = cst.tile([P, P], f32)
            make_identity(nc, ident[:])
            iota_i = cst.tile([P, P], i32)
            nc.gpsimd.iota(iota_i[:], pattern=[[1, P]], base=0, channel_multiplier=0)
            iota_b = cst.tile([P, P], bf16)
            nc.vector.tensor_copy(out=iota_b[:], in_=iota_i[:])
            g16_sb = cst.tile([32, NHS * 8 * P], mybir.dt.int16)
            nc.sync.dma_start(out=g16_sb[:], in_=g16[:, :])
            govf_sb = cst.tile([P, CH * B_OV], i32)
            nc.sync.dma_start(out=govf_sb[:], in_=govf[:, :])
            gdst_sb = cst.tile([P, NB], bf16)
            nc.sync.dma_start(out=gdst_sb[:], in_=gdst[:, :])
            As_sb = cst.tile([P, NT, SGL], bf16)
            nc.sync.dma_start(out=As_sb[:], in_=Asn[:, :].rearrange(
                "(t p) j -> p t j", p=P))
            eps_col = cst.tile([P, 1], f32)
            nc.vector.memset(eps_col[:], EPS)
            hsub = cst.tile([P, CH], f32)

            params = {}

            for li in range(L):
                z2_full, xsum_full = z2_fulls[li], xsum_fulls[li]
                wsl = slice(li * P, (li + 1) * P)
                Wn_t = wts.tile([P, P], bf16, tag="Wn_t")
                nc.sync.dma_start(out=Wn_t[:], in_=Wn[wsl, :])
                Wr_t = wts.tile([P, P], bf16, tag="Wr_t")
                nc.sync.dma_start(out=Wr_t[:], in_=Wr[wsl, :])
                Wns_t = wts.tile([P, P], bf16, tag="Wns_t")
                nc.sync.dma_start(out=Wns_t[:], in_=Wns[wsl, :])
                Wrs_t = wts.tile([P, P], bf16, tag="Wrs_t")
                nc.sync.dma_start(out=Wrs_t[:], in_=Wrs[wsl, :])
                vecs = {}
                for nm, src in (("b", bia), ("g", gam), ("be", bet),
                                ("bs", bias_), ("gs", gams), ("bes", bets)):
                    v = wts.tile([P, 1], f32, tag=f"v_{nm}")
                    nc.sync.dma_start(out=v[:], in_=src[wsl, :])
                    vecs[nm] = v

                h1_cur = (h1_a if li % 2 == 0 else h1_b)
                h1_prev = (h1_b if li % 2 == 0 else h1_a)

                # ---------- pass 1: z2 + x_sum + piecewise AllGather ----------
                xsum_acc = sm.tile([P, SGL], f32, tag="xsum_acc")
                nc.vector.memset(xsum_acc[:], 0.0)
                for sl in range(NSLAB):
                    gl = sl // (NSLAB // G_loc)
                    raw = io.tile([P, SLAB], bf16, tag="raw")
                    src = xT0 if li == 0 else h1_prev[:]
                    nc.sync.dma_start(out=raw[:],
                                      in_=src[:, sl * SLAB:(sl + 1) * SLAB])
                    if li > 0:
                        A1p, Cgp = params[li - 1]
                        xt = io.tile([P, SLAB], bf16, tag="xt")
                        nc.vector.tensor_scalar_mul(out=xt[:], in0=raw[:],
                                                    scalar1=A1p[:])
                        nc.vector.tensor_tensor(
                            out=xt[:].rearrange("p (a b) -> p a b", a=CPS),
                            in0=xt[:].rearrange("p (a b) -> p a b", a=CPS),
                            in1=rep3(Cgp[:, gl * P:(gl + 1) * P], CPS, P, False),
                            op=mybir.AluOpType.add)
                        nc.vector.tensor_scalar_max(out=xt[:], in0=xt[:],
                                                    scalar1=0.0)
                        nc.sync.dma_start(out=x_aff[:, sl * SLAB:(sl + 1) * SLAB],
                                          in_=xt[:])
                    else:
                        xt = raw
                    z2st = io.tile([P, CPS, P], bf16, tag="z2st")
                    for j in range(CPS):
                        pz = ps_t.tile([P, P], f32, tag="pz")
                        nc.tensor.matmul(out=pz[:], lhsT=xt[:, j * P:(j + 1) * P],
                                         rhs=Wn_t[:], start=True, stop=True)
                        nc.scalar.activation(out=z2st[:, j, :], in_=pz[:],
                                             func=mybir.ActivationFunctionType.Copy)
                    nc.sync.dma_start(
                        out=z2_stage[:].rearrange("(a j p) f -> p (a j) f",
                                                  p=P, j=CPS)[
                            :, sl * CPS:(sl + 1) * CPS, :],
                        in_=z2st[:])
                    red = sm.tile([P, P], f32, tag="red")
                    nc.vector.tensor_reduce(
                        out=red[:], in_=xt[:].rearrange("p (s n) -> p n s", s=CPS),
                        axis=mybir.AxisListType.X, op=mybir.AluOpType.add)
                    nc.vector.tensor_add(out=xsum_acc[:, gl * P:(gl + 1) * P],
                                         in0=xsum_acc[:, gl * P:(gl + 1) * P],
                                         in1=red[:])
                nc.gpsimd.collective_compute(
                    "AllGather", mybir.AluOpType.bypass, replica_groups=rg,
                    ins=[z2_stage[:]], outs=[z2_full[:]])
                if taps and li == 0:
                    nc.sync.dma_start(out=dz2[:, :], in_=z2_stage[:])

                # x_sum scaled (own cols, feature-major) + AllGather
                xsown = sm.tile([P, SGL], bf16, tag="xsown")
                nc.scalar.activation(out=xsown[:], in_=xsum_acc[:],
                                     func=mybir.ActivationFunctionType.Copy,
                                     scale=1.0 / cfg.N)
                nc.sync.dma_start(out=xsum_bounce[:, :], in_=xsown[:])
                nc.gpsimd.collective_compute(
                    "AllGather", mybir.AluOpType.bypass, replica_groups=rg,
                    ins=[xsum_bounce[:]], outs=[xsum_full[:]])

                # ---------- h2 path (own columns only) ----------
                xsT = sm.tile([P, NC, SGL], bf16, tag="xsT")
                nc.sync.dma_start(out=xsT[:], in_=xsum_full[:].rearrange(
                    "(k p) j -> p k j", p=P))
                zs = sm.tile([P, NT, P], bf16, tag="zs")
                for t in range(NT):
                    kt, ht = t // G_loc, t % G_loc
                    pz = ps_t.tile([P, P], f32, tag="pz")
                    nc.tensor.matmul(out=pz[:],
                                     lhsT=xsT[:, kt, ht * P:(ht + 1) * P],
                                     rhs=Wns_t[:], start=True, stop=True)
                    nc.scalar.activation(out=zs[:, t, :], in_=pz[:],
                                         func=mybir.ActivationFunctionType.Copy)
                ph = ps_h.tile([P, SGL], f32, tag="ph")
                nc.tensor.matmul(out=ph[:], lhsT=Wrs_t[:], rhs=xsown[:],
                                 start=True, stop=False)
                for t in range(NT):
                    nc.tensor.matmul(out=ph[:], lhsT=zs[:, t, :],
                                     rhs=As_sb[:, t, :],
                                     start=False, stop=(t == NT - 1))
                h2pre = sm.tile([P, SGL], f32, tag="h2pre")
                nc.scalar.activation(out=h2pre[:], in_=ph[:],
                                     func=mybir.ActivationFunctionType.Identity,
                                     bias=vecs["bs"][:])
                # BN2 stats (partial over own cols) -> AllReduce
                stv2 = sm.tile([P, 2], f32, tag="stv2")
                nc.vector.tensor_reduce(out=stv2[:, 0:1], in_=h2pre[:],
                                        axis=mybir.AxisListType.X,
                                        op=mybir.AluOpType.add)
                sq2 = sm.tile([P, SGL], f32, tag="sq2")
                nc.vector.tensor_tensor(out=sq2[:], in0=h2pre[:], in1=h2pre[:],
                                        op=mybir.AluOpType.mult)
                nc.vector.tensor_reduce(out=stv2[:, 1:2], in_=sq2[:],
                                        axis=mybir.AxisListType.X,
                                        op=mybir.AluOpType.add)
                nc.sync.dma_start(out=st_bounce[:], in_=stv2[:])
                nc.gpsimd.collective_compute(
                    "AllReduce", mybir.AluOpType.add, replica_groups=rg,
                    ins=[st_bounce[:]], outs=[st_reds[2 * li][:]])
                str2 = sm.tile([P, 2], f32, tag="str2")
                nc.sync.dma_start(out=str2[:], in_=st_reds[2 * li][:])

                def bn_coeffs(str_, n, gamma_v, beta_v, tag):
                    mu = sm.tile([P, 1], f32, tag=f"mu{tag}")
                    nc.vector.tensor_scalar_mul(out=mu[:], in0=str_[:, 0:1],
                                                scalar1=1.0 / n)
                    ex = sm.tile([P, 1], f32, tag=f"ex{tag}")
                    nc.vector.tensor_scalar_mul(out=ex[:], in0=str_[:, 1:2],
                                                scalar1=1.0 / n)
                    mus = sm.tile([P, 1], f32, tag=f"mus{tag}")
                    nc.vector.tensor_tensor(out=mus[:], in0=mu[:], in1=mu[:],
                                            op=mybir.AluOpType.mult)
                    var = sm.tile([P, 1], f32, tag=f"var{tag}")
                    nc.vector.tensor_tensor(out=var[:], in0=ex[:], in1=mus[:],
                                            op=mybir.AluOpType.subtract)
                    sd = sm.tile([P, 1], f32, tag=f"sd{tag}")
                    nc.scalar.activation(out=sd[:], in_=var[:],
                                         func=mybir.ActivationFunctionType.Sqrt,
                                         bias=eps_col[:])
                    rstd = sm.tile([P, 1], f32, tag=f"rstd{tag}")
                    nc.vector.reciprocal(out=rstd[:], in_=sd[:])
                    A = sm.tile([P, 1], f32, tag=f"A{tag}")
                    nc.vector.tensor_tensor(out=A[:], in0=gamma_v[:], in1=rstd[:],
                                            op=mybir.AluOpType.mult)
                    muA = sm.tile([P, 1], f32, tag=f"muA{tag}")
                    nc.vector.tensor_tensor(out=muA[:], in0=mu[:], in1=A[:],
                                            op=mybir.AluOpType.mult)
                    C = sm.tile([P, 1], f32, tag=f"C{tag}")
                    nc.vector.tensor_tensor(out=C[:], in0=beta_v[:], in1=muA[:],
                                            op=mybir.AluOpType.subtract)
                    return A, C

                A2, C2 = bn_coeffs(str2, S, vecs["gs"], vecs["bes"], "2")
                h2own = sm.tile([P, SGL], f32, tag="h2own")
                nc.vector.tensor_scalar(out=h2own[:], in0=h2pre[:],
                                        scalar1=A2[:], scalar2=C2[:],
                                        op0=mybir.AluOpType.mult,
                                        op1=mybir.AluOpType.add)
                if taps and li == 0:
                    nc.sync.dma_start(out=dxs[:, :], in_=xsum_bounce[:])
                    nc.sync.dma_start(out=dh2[:, :], in_=h2own[:])
                    nc.sync.dma_start(out=dst_[:, 0:2], in_=str2[:])

                # ---------- pass 2: batched gather + scatter ----------
                ssum = sm.tile([P, NHS], f32, tag="ssum")
                ssq = sm.tile([P, NHS], f32, tag="ssq")
                x2src = xT0 if li == 0 else x_aff[:]
                for hs in range(NHS):
                    xa = io.tile([P, 16 * P], bf16, tag="xa")
                    nc.sync.dma_start(out=xa[:],
                                      in_=x2src[:, hs * 16 * P:(hs + 1) * 16 * P])
                    xgs = xsp.tile([P, 8, 16, P], bf16, tag="xgs")
                    for w in range(NC):
                        nc.gpsimd.dma_gather(
                            out_ap=xgs[:, w, :, :],
                            in_ap=z2_full[w * TP:(w + 1) * TP, :],
                            idxs_ap=g16_sb[:, (hs * 8 + w) * P:(hs * 8 + w + 1) * P],
                            num_idxs=16 * P, num_idxs_reg=16 * P,
                            elem_size=P, single_packet=False)
                    h1st = io.tile([P, 16, P], bf16, tag="h1st")
                    for ci in range(16):
                        c = hs * 16 + ci
                        oh = ohp.tile([P, B, P], bf16, tag="oh")
                        nc.vector.tensor_tensor(
                            out=oh[:],
                            in0=rep3(gdst_sb[:, c * B:(c + 1) * B], B, P, True),
                            in1=rep3(iota_b[:], B, P, False),
                            op=mybir.AluOpType.is_equal)
                        pm = ps_m.tile([P, P], f32, tag="pm")
                        nc.tensor.matmul(out=pm[:], lhsT=Wr_t[:],
                                         rhs=xa[:, ci * P:(ci + 1) * P],
                                         start=True, stop=False)
                        for w in range(NC):
                            nc.tensor.matmul(out=pm[:],
                                             lhsT=xgs[:, w, ci, :],
                                             rhs=oh[:, w, :],
                                             start=False, stop=False)
                        for b in range(B_OV):
                            xg = xgp.tile([P, P], bf16, tag="xg")
                            nc.gpsimd.indirect_dma_start(
                                out=xg[:], out_offset=None, in_=z2_full[:],
                                in_offset=bass.IndirectOffsetOnAxis(
                                    ap=govf_sb[:, c * B_OV + b:c * B_OV + b + 1],
                                    axis=0))
                            nc.tensor.matmul(out=pm[:],
                                             lhsT=xg[:], rhs=oh[:, 8 + b, :],
                                             start=False, stop=(b == B_OV - 1))
                        nc.scalar.activation(out=h1st[:, ci, :], in_=pm[:],
                                             func=mybir.ActivationFunctionType.Identity,
                                             bias=vecs["b"][:])
                    nc.sync.dma_start(out=h1_cur[:, hs * 16 * P:(hs + 1) * 16 * P],
                                      in_=h1st[:].rearrange("p a b -> p (a b)"))
                    rs = sm.tile([P, 1], f32, tag="rs")
                    nc.vector.tensor_reduce(
                        out=rs[:], in_=h1st[:].rearrange("p a b -> p (a b)"),
                        axis=mybir.AxisListType.X, op=mybir.AluOpType.add)
                    nc.vector.tensor_copy(out=ssum[:, hs:hs + 1], in_=rs[:])
                    sqt = io.tile([P, 16 * P], bf16, tag="sqt")
                    nc.vector.tensor_tensor(
                        out=sqt[:], in0=h1st[:].rearrange("p a b -> p (a b)"),
                        in1=h1st[:].rearrange("p a b -> p (a b)"),
                        op=mybir.AluOpType.mult)
                    rq = sm.tile([P, 1], f32, tag="rq")
                    nc.vector.tensor_reduce(out=rq[:], in_=sqt[:],
                                            axis=mybir.AxisListType.X,
                                            op=mybir.AluOpType.add)
                    nc.vector.tensor_copy(out=ssq[:, hs:hs + 1], in_=rq[:])

                # ---------- BN1 stats AllReduce + next-layer params ----------
                stv1 = sm.tile([P, 2], f32, tag="stv1")
                nc.vector.tensor_reduce(out=stv1[:, 0:1], in_=ssum[:],
                                        axis=mybir.AxisListType.X,
                                        op=mybir.AluOpType.add)
                nc.vector.tensor_reduce(out=stv1[:, 1:2], in_=ssq[:],
                                        axis=mybir.AxisListType.X,
                                        op=mybir.AluOpType.add)
                nc.sync.dma_start(out=st_bounce[:], in_=stv1[:])
                nc.gpsimd.collective_compute(
                    "AllReduce", mybir.AluOpType.add, replica_groups=rg,
                    ins=[st_bounce[:]], outs=[st_reds[2 * li + 1][:]])
                str1 = sm.tile([P, 2], f32, tag="str1")
                nc.sync.dma_start(out=str1[:], in_=st_reds[2 * li + 1][:])
                A1, sh1 = bn_coeffs(str1, T, vecs["g"], vecs["be"], "1")
                A1_sb = prm.tile([P, 1], f32, tag=f"A1_{li}")
                nc.vector.tensor_copy(out=A1_sb[:], in_=A1[:])
                Cg_sb = prm.tile([P, SGL], bf16, tag=f"Cg_{li}")
                nc.vector.tensor_scalar_add(out=Cg_sb[:], in0=h2own[:],
                                            scalar1=sh1[:])
                params[li] = (A1_sb, Cg_sb)
                if taps and li == 0:
                    nc.sync.dma_start(out=dh1[:, :], in_=h1_cur[:])
                    nc.sync.dma_start(out=dcg[:, :], in_=Cg_sb[:])
                    nc.sync.dma_start(out=dst_[:, 2:4], in_=str1[:])
                if taps and li == 1:
                    nc.sync.dma_start(out=dxa[:, :], in_=x_aff[:])

            # ---------- readout ----------
            A1f, Cgf = params[L - 1]
            h1_fin = (h1_a if (L - 1) % 2 == 0 else h1_b)
            for sl in range(NSLAB):
                gl = sl // (NSLAB // G_loc)
                raw = io.tile([P, SLAB], bf16, tag="raw")
                nc.sync.dma_start(out=raw[:],
                                  in_=h1_fin[:, sl * SLAB:(sl + 1) * SLAB])
                xt = io.tile([P, SLAB], bf16, tag="xt")
                nc.vector.tensor_scalar_mul(out=xt[:], in0=raw[:], scalar1=A1f[:])
                nc.vector.tensor_tensor(
                    out=xt[:].rearrange("p (a b) -> p a b", a=CPS),
                    in0=xt[:].rearrange("p (a b) -> p a b", a=CPS),
                    in1=rep3(Cgf[:, gl * P:(gl + 1) * P], CPS, P, False),
                    op=mybir.AluOpType.add)
                nc.vector.tensor_scalar_max(out=xt[:], in0=xt[:], scalar1=0.0)
                nc.vector.tensor_reduce(
                    out=hsub[:, sl * CPS:(sl + 1) * CPS],
                    in_=xt[:].rearrange("p (c n) -> p c n", c=CPS),
                    axis=mybir.AxisListType.X, op=mybir.AluOpType.add)
            hg = sm.tile([P, G_loc], f32, tag="hg")
            nc.vector.tensor_reduce(
                out=hg[:], in_=hsub[:].rearrange("p (g s) -> p g s", g=G_loc),
                axis=mybir.AxisListType.X, op=mybir.AluOpType.add)
            nc.vector.tensor_scalar_mul(out=hg[:], in0=hg[:],
                                        scalar1=1.0 / (cfg.N * cfg.N))
            Wf1_sb = sm.tile([P, 2 * P], f32, tag="Wf1_sb")
            nc.sync.dma_start(out=Wf1_sb[:], in_=Wf1[:, :])
            Wf2_sb = sm.tile([P, 2, TASKS], f32, tag="Wf2_sb")
            nc.sync.dma_start(out=Wf2_sb[:],
                              in_=Wf2[:, :].rearrange("(a p) t -> p a t", p=P))
            bf1_sb = sm.tile([P, 2], f32, tag="bf1_sb")
            nc.sync.dma_start(out=bf1_sb[:],
                              in_=bf1c[:, :].rearrange("(a p) o -> p (a o)", p=P))
            bf2_sb = sm.tile([TASKS, 1], f32, tag="bf2_sb")
            nc.sync.dma_start(out=bf2_sb[:], in_=bf2c[:, :])
            o1 = sm.tile([P, 2, G_loc], f32, tag="o1")
            for h in range(2):
                p1 = ps_t.tile([P, G_loc], f32, tag="pz")
                nc.tensor.matmul(out=p1[:], lhsT=Wf1_sb[:, h * P:(h + 1) * P],
                                 rhs=hg[:], start=True, stop=True)
                nc.scalar.activation(out=o1[:, h, :], in_=p1[:],
                                     func=mybir.ActivationFunctionType.Relu,
                                     bias=bf1_sb[:, h:h + 1])
            p2 = ps_t.tile([TASKS, G_loc], f32, tag="pz")
            for h in range(2):
                nc.tensor.matmul(out=p2[:], lhsT=Wf2_sb[:, h, :], rhs=o1[:, h, :],
                                 start=(h == 0), stop=(h == 1))
            oT = sm.tile([TASKS, G_loc], f32, tag="oT")
            nc.scalar.activation(out=oT[:], in_=p2[:],
                                 func=mybir.ActivationFunctionType.Identity,
                                 bias=bf2_sb[:])
            nc.sync.dma_start(out=out[:, :].rearrange("a b -> b a"), in_=oT[:])

    nc.compile()
    return nc


def host_prep(inputs, cfg: Cfg):
    """Full inputs dict -> in_maps list per core."""
    import ml_dtypes
    bf = ml_dtypes.bfloat16
    NC, G, N, L, S, T, TP = cfg.NC, cfg.G, cfg.N, cfg.L, cfg.S, cfg.T, cfg.TP
    x = np.asarray(inputs["x"], np.float32)
    ei = np.asarray(inputs["edge_index"])
    oei = np.asarray(inputs["original_edge_index"])
    batch = np.asarray(inputs["batch"])
    sni = np.asarray(inputs["subgraph_node_idx"])
    sb = np.asarray(inputs["subgraph_batch"])
    nnps = np.asarray(inputs["num_nodes_per_subgraph"])
    sib = np.asarray(inputs["subgraph_idx_batch"])

    # verify the structured DSS layout this kernel hardcodes
    assert np.array_equal(batch, np.repeat(np.arange(G), N * N))
    assert np.array_equal(sni, np.tile(np.arange(N), S))
    assert np.array_equal(sb, np.repeat(np.arange(S), N))
    assert np.all(nnps == N)
    assert np.array_equal(sib, np.repeat(np.arange(G), N))

    As = np.zeros((S, S), np.float32)
    np.add.at(As, (oei[0], oei[1]), 1.0)

    src, dst = ei[0].astype(np.int64), ei[1].astype(np.int64)
    core = dst // TP

    # pass 1: per-core (chunk, window) split; size the overflow capacity
    per_core = []
    max_ov = 0
    for k in range(NC):
        sel = core == k
        s_g = src[sel]
        d_k = dst[sel] - k * TP
        w_k = s_g // TP
        loc = s_g % TP
        ch = d_k // 128
        key = ch * NC + w_k
        order = np.argsort(key, kind="stable")
        key_o = key[order]
        gstart = np.concatenate(
            [[0], np.cumsum(np.bincount(key_o, minlength=cfg.CH * NC))])[:-1]
        rank = np.arange(len(key_o)) - gstart[key_o]
        main = rank < 128
        ovc = np.bincount(ch[order][~main], minlength=cfg.CH)
        max_ov = max(max_ov, int(ovc.max()) if ovc.size else 0)
        per_core.append((ch[order], w_k[order], loc[order], s_g[order],
                         (d_k % 128)[order], rank, main))
    B_OV = max(1, int(np.ceil(max_ov / 128)))
    B = 8 + B_OV
    cfg.B = B
    cfg.NB = cfg.CH * B
    NHS = cfg.CH // 16

    def stack(w):
        return np.asarray(w, np.float32).reshape(L * 128, 128)

    def col(v):
        return np.asarray(v, np.float32).reshape(L * 128, 1)

    common = dict(
        Wr=stack(inputs["Wr"]).astype(bf), Wn=stack(inputs["Wn"]).astype(bf),
        Wrs=stack(inputs["Wr_s"]).astype(bf), Wns=stack(inputs["Wn_s"]).astype(bf),
        bia=col(inputs["b"]), gam=col(inputs["gamma"]), bet=col(inputs["beta"]),
        bias_=col(inputs["b_s"]), gams=col(inputs["gamma_s"]),
        bets=col(inputs["beta_s"]),
        Wf1=np.asarray(inputs["Wf1"], np.float32),
        bf1c=np.asarray(inputs["bf1"], np.float32).reshape(2 * 128, 1),
        Wf2=np.asarray(inputs["Wf2"], np.float32),
        bf2c=np.asarray(inputs["bf2"], np.float32).reshape(cfg.TASKS, 1),
    )

    in_maps = []
    for k in range(NC):
        ch_o, w_o, loc_o, glob_o, dp_o, rank, main = per_core[k]
        win_idx = np.zeros((cfg.CH, NC, 128), np.int16)      # pad = row 0
        win_dst = np.full((cfg.CH, NC, 128), -1.0, np.float32)
        win_idx[ch_o[main], w_o[main], rank[main]] = loc_o[main].astype(np.int16)
        win_dst[ch_o[main], w_o[main], rank[main]] = dp_o[main]
        ovm = ~main
        och, og, od = ch_o[ovm], glob_o[ovm], dp_o[ovm]
        ostart = np.concatenate(
            [[0], np.cumsum(np.bincount(och, minlength=cfg.CH))])[:-1]
        orank = np.arange(len(och)) - ostart[och]
        ov_idx = np.zeros((cfg.CH, B_OV * 128), np.int32)    # pad = row 0
        ov_dst = np.full((cfg.CH, B_OV * 128), -1.0, np.float32)
        ov_idx[och, orank] = og
        ov_dst[och, orank] = od

        g16a = np.zeros((32, NHS * 8 * 128), np.int16)
        for hs in range(NHS):
            for w in range(NC):
                seg = win_idx[hs * 16:(hs + 1) * 16, w, :].reshape(16 * 128)
                arr = seg.reshape(128, 16).T                 # [16, 128]
                colb = (hs * 8 + w) * 128
                g16a[0:16, colb:colb + 128] = arr
                g16a[16:32, colb:colb + 128] = arr
        govfa = np.ascontiguousarray(
            ov_idx.reshape(cfg.CH * B_OV, 128).T).astype(np.int32)
        gd = np.concatenate(
            [win_dst.reshape(cfg.CH, NC * 128), ov_dst], axis=1)
        gd = np.ascontiguousarray(
            gd.reshape(cfg.CH * B, 128).T).astype(bf)

        m = dict(common)
        m.update(
            xT0=np.ascontiguousarray(x[k * TP:(k + 1) * TP].T).astype(bf),
            g16=g16a, govf=govfa, gdst=gd,
            Asn=np.ascontiguousarray(
                As[:, k * cfg.SGL:(k + 1) * cfg.SGL]).astype(bf),
        )
        in_maps.append(m)
    return in_maps


_CACHE = {}


def kernel(**inputs):
    """Full (unsharded) inputs -> full [G, TASKS] output, computed on 8
    trn2 NeuronCores via bass."""
    from concourse import bass_utils

    G = int(np.asarray(inputs["num_nodes_per_subgraph"]).shape[0])
    N = int(np.asarray(inputs["num_nodes_per_subgraph"])[0])
    TASKS = int(np.asarray(inputs["bf2"]).shape[0])
    L = int(np.asarray(inputs["Wr"]).shape[0])
    NC = 8

    cfg = Cfg(NC=NC, G=G, N=N, L=L, TASKS=TASKS)
    in_maps = host_prep(inputs, cfg)

    key = (NC, G, N, L, TASKS, cfg.B)
    if key not in _CACHE:
        _CACHE[key] = build(cfg)
    nc = _CACHE[key]

    res = bass_utils.run_bass_kernel_spmd(
        nc, in_maps, core_ids=list(range(NC)), trace=False)
    out = np.concatenate([res.results[k]["out"] for k in range(NC)], axis=0)
    return out.astype(np.float32)
